# revision 1
# baseline (speedup 1.0000x reference)
"""Bass/Trainium2 kernel for the additive-attention nn.Module.

Computation (per batch b):
    energy[s, a] = tanh( enc[b,s,:] @ W_enc[a,:] + hidden[b,:] @ W_dec[a,:] + b_attn[a] )
    scores[s]    = energy[s, :] @ v
    w            = softmax(scores)
    ctx[b, :]    = w @ enc[b]

Sharding: data-parallel over batch across 8 NeuronCores (8 batches/core);
W_attn / b_attn / v replicated.

Per-core kernel layout:
  - W_attn is transposed on-chip (PE transposes) into W_encT [e, a] tiles kept
    in SBUF; the W_dec half is consumed on the fly by the tiny hidden GEMM,
    which together with b_attn produces a per-(a, b) bias table hb.
  - The big GEMM runs as out[a(128), t(512)] += W_encT[e,a].T @ encT[e,t] in
    float32r (fp32 data at 1 PE cycle/row; all operand tiles are float32r-
    typed so producers round on write, as the BIR verifier requires).
  - ScalarEngine computes tanh(energy + hb[:, b]) in one op (per-partition
    bias), then PE contracts with v (partition-dim reduction) into scores.
  - Per-batch softmax on one SBUF row; the exp-weights are re-laid into PE
    columns via a tiny DRAM bounce, then ctx = (ew @ enc) * (1/sum) using the
    natural-layout enc tiles kept resident in SBUF (no HBM re-read).
"""

import sys

if "/opt/trn_rl_repo" not in sys.path:
    sys.path.insert(0, "/opt/trn_rl_repo")

import numpy as np

B, S, DEC, ENC, ATTN = 64, 1024, 1024, 1024, 1024
N_CORES = 8
B_LOC = B // N_CORES

_CACHE = {}


def build_nc(B_loc=B_LOC, S_=S, E=ENC, A=ATTN, D=DEC, loop_n=None):
    from contextlib import ExitStack

    import concourse.bacc as bacc
    import concourse.tile as tile
    from concourse import mybir
    from concourse.bass import ts
    from concourse.masks import make_identity

    P = 128
    F32 = mybir.dt.float32
    F32R = mybir.dt.float32r
    AF = mybir.ActivationFunctionType
    AX = mybir.AxisListType

    n_tb = S_ // P            # 128-token blocks per batch
    TCW = min(512, S_)        # token-chunk width (matmul moving N)
    n_tc = S_ // TCW
    tb_per_tc = TCW // P
    n_eb = E // P
    n_ab = A // P
    n_db = D // P
    ECW = min(512, E)         # e-chunk width for the context matmul
    n_ec = E // ECW
    AB_GRP = min(2, n_ab)     # a-blocks packed per transpose-psum tile
    n_abg = n_ab // AB_GRP
    F = D + E

    nc = bacc.Bacc("TRN2", target_bir_lowering=False, debug=False)
    hid_d = nc.dram_tensor("hidden", [B_loc, D], F32, kind="ExternalInput")
    enc_d = nc.dram_tensor("enc", [B_loc, S_, E], F32, kind="ExternalInput")
    W_d = nc.dram_tensor("W", [A, F], F32, kind="ExternalInput")
    b_d = nc.dram_tensor("b_attn", [A], F32, kind="ExternalInput")
    v_d = nc.dram_tensor("v", [A], F32, kind="ExternalInput")
    ctx_d = nc.dram_tensor("ctx", [B_loc, E], F32, kind="ExternalOutput")

    with tile.TileContext(nc) as tc:
        with ExitStack() as ctx:
            const = ctx.enter_context(tc.tile_pool(name="const", bufs=1))
            wnat_p = ctx.enter_context(tc.tile_pool(name="wnat", bufs=3 * AB_GRP))
            wenc_p = ctx.enter_context(tc.tile_pool(name="wenc", bufs=1))
            wdec_p = ctx.enter_context(tc.tile_pool(name="wdec", bufs=n_db + 1))
            encnat_p = ctx.enter_context(tc.tile_pool(name="encnat", bufs=min(16, B_loc * n_tb)))
            encT_p = ctx.enter_context(tc.tile_pool(name="encT", bufs=2 * n_eb))
            tanh_p = ctx.enter_context(tc.tile_pool(name="tanh", bufs=6))
            soft_p = ctx.enter_context(tc.tile_pool(name="soft", bufs=2))
            psE = ctx.enter_context(tc.tile_pool(name="psE", bufs=3, space="PSUM"))
            psT = ctx.enter_context(tc.tile_pool(name="psT", bufs=2, space="PSUM"))
            psS = ctx.enter_context(tc.tile_pool(name="psS", bufs=1, space="PSUM"))
            psC = ctx.enter_context(tc.tile_pool(name="psC", bufs=1, space="PSUM"))
            psH = ctx.enter_context(tc.tile_pool(name="psH", bufs=1, space="PSUM"))

            if loop_n is not None:
                ctx.enter_context(tc.For_i(0, loop_n, 1))

            # ---- constants ----
            ident0 = const.tile([P, P], F32, name="ident0")
            make_identity(nc, ident0[:])
            ident = const.tile([P, P], F32R, name="ident")
            nc.vector.tensor_copy(ident[:], ident0[:])
            encnat_all = {}

            def load_enc(b, tb):
                t_enc = encnat_p.tile([P, E], F32R, tag="encnat", name=f"enc_{b}_{tb}")
                nc.sync.dma_start(t_enc[:], enc_d.ap()[b, ts(tb, P), :].bitcast(F32R))
                encnat_all[(b, tb)] = t_enc

            # issue the first W row-block loads ahead of everything (the first
            # energy matmuls are gated on W_encT availability), then prefetch
            # the first pairs' enc tiles to overlap the rest of W setup
            wn_all = {}

            def load_wn(ab, half):
                # half 1 = W_enc columns (feeds energy GEMM), half 0 = W_dec
                lo, width = (D, E) if half else (0, D)
                wn = wnat_p.tile([P, width], F32R, tag="wnat", name=f"wn{ab}_{half}")
                nc.sync.dma_start(
                    wn[:], W_d.ap()[ts(ab, P), lo:lo + width].bitcast(F32R)
                )
                wn_all[(ab, half)] = wn

            for ab in range(min(AB_GRP + 1, n_ab)):
                load_wn(ab, 1)

            _pairs0 = [(b, tcn) for b in range(B_loc) for tcn in range(n_tc)][:2]
            for b0, tc0 in _pairs0:
                for tbl in range(tb_per_tc):
                    tb0 = tc0 * tb_per_tc + tbl
                    if (b0, tb0) not in encnat_all:
                        load_enc(b0, tb0)

            # small gathers issued after the critical W/enc loads
            ones_row = const.tile([1, B_loc], F32, name="ones_row")
            nc.gpsimd.memset(ones_row[:], 1.0)
            ones1 = const.tile([1, 1], F32, name="ones1")
            nc.gpsimd.memset(ones1[:], 1.0)
            b_row = const.tile([1, A], F32, name="b_row")
            nc.sync.dma_start(b_row[:], b_d.ap().rearrange("(o a) -> o a", o=1))
            vcol = const.tile([P, n_ab], F32R, name="vcol")
            nc.sync.dma_start(vcol[:], v_d.ap().rearrange("(j p) -> p j", p=P).bitcast(F32R))
            # hidden as [d, db, b] columns, straight from DRAM
            hidT = const.tile([P, n_db, B_loc], F32R, name="hidT")
            for db in range(n_db):
                nc.sync.dma_start(
                    hidT[:, db],
                    hid_d.ap()[:, ts(db, P)].rearrange("b p -> p b").bitcast(F32R),
                )

            # ---- W transpose + hb[a, b] = W_dec @ hidden.T + b_attn ----
            # wenc tiles split per (eb, abg) so early a-blocks unblock ASAP
            wenc_t = {}
            for eb in range(n_eb):
                for abg in range(n_abg):
                    wenc_t[(eb, abg)] = wenc_p.tile(
                        [P, AB_GRP * P], F32R, tag=f"wenc{eb}_{abg}", name=f"wenc{eb}_{abg}"
                    )
            hb_all = const.tile([P, n_ab * B_loc], F32, name="hb_all")

            def emit_w_abg(abg):
                for abl in range(AB_GRP):
                    ab = abg * AB_GRP + abl
                    for half in (1, 0):
                        if (ab, half) not in wn_all:
                            load_wn(ab, half)
                wd_strips = {}
                # W_enc f-blocks first (unblock energy matmuls), then W_dec
                for fb in list(range(n_db, n_db + n_eb)) + list(range(n_db)):
                    half = 1 if fb >= n_db else 0
                    col = fb - n_db if fb >= n_db else fb
                    pw = psT.tile([P, AB_GRP * P], F32R, tag="t", name=f"pw{abg}_{fb}")
                    for abl in range(AB_GRP):
                        ab = abg * AB_GRP + abl
                        nc.tensor.transpose(
                            pw[:, ts(abl, P)],
                            wn_all[(ab, half)][:, ts(col, P)],
                            ident[:],
                        )
                    if fb < n_db:
                        db = fb
                        wd = wdec_p.tile([P, AB_GRP * P], F32R, tag="wdec", name=f"wd{abg}_{db}")
                        nc.vector.tensor_copy(wd[:], pw[:])
                        wd_strips[db] = wd
                    else:
                        eb = fb - n_db
                        if eb % 2 == 0:
                            nc.vector.tensor_copy(wenc_t[(eb, abg)][:], pw[:])
                        else:
                            nc.scalar.copy(wenc_t[(eb, abg)][:], pw[:])
                # hb accumulation, one a-block at a time (single PSUM bank)
                for abl in range(AB_GRP):
                    ab = abg * AB_GRP + abl
                    ps_hb = psH.tile([P, B_loc], F32, tag="hb", name=f"pshb{abg}_{abl}")
                    for db in range(n_db):
                        nc.tensor.matmul(
                            ps_hb[:],
                            wd_strips[db][:, ts(abl, P)],
                            hidT[:, db],
                            start=(db == 0),
                            stop=False,
                            skip_group_check=True,
                        )
                    nc.tensor.matmul(
                        ps_hb[:],
                        b_row[0:1, ts(ab, P)],
                        ones_row[:],
                        start=False,
                        stop=True,
                        skip_group_check=True,
                    )
                    nc.vector.tensor_copy(
                        hb_all[:, ab * B_loc:(ab + 1) * B_loc], ps_hb[:]
                    )

            w_emitted = set()

            def ensure_w(abg):
                if abg not in w_emitted:
                    w_emitted.add(abg)
                    emit_w_abg(abg)

            for _abg in range(min(2, n_abg)):
                ensure_w(_abg)

            # ---- main loop (software-pipelined emission) ----
            # per pair i: energy/tanh/scores for pair i, interleaved with the
            # transposes for pair i+1 and the deferred context matmuls of the
            # previous batch, so the PE queue never runs dry at batch edges.
            pairs = [(b, tcn) for b in range(B_loc) for tcn in range(n_tc)]
            encT_cur: list = []
            encT_next: list = []
            scores_rows = {}
            pending_ctx = None  # (b, wc, rc, ctx_row)

            def emit_transposes(b, tcn, eb):
                pt = psT.tile([P, TCW], F32R, tag="t", name=f"pt{b}_{tcn}_{eb}")
                for tbl in range(tb_per_tc):
                    tb = tcn * tb_per_tc + tbl
                    nc.tensor.transpose(
                        pt[:, ts(tbl, P)], encnat_all[(b, tb)][:, ts(eb, P)], ident[:]
                    )
                et = encT_p.tile([P, TCW], F32R, tag="encT", name=f"encT{b}_{tcn}_{eb}")
                if eb % 2 == 0:
                    nc.vector.tensor_copy(et[:], pt[:])
                else:
                    nc.scalar.copy(et[:], pt[:])
                return et

            def emit_ctx_chunk(bb, wc, rc, ctx_row, ec):
                ps_c = psC.tile([1, ECW], F32, tag="c", name=f"psc{bb}_{ec}")
                for tb in range(n_tb):
                    nc.tensor.matmul(
                        ps_c[:],
                        wc[:, tb:tb + 1],
                        encnat_all[(bb, tb)][:, ts(ec, ECW)],
                        start=(tb == 0),
                        stop=(tb == n_tb - 1),
                        skip_group_check=True,
                    )
                nc.vector.tensor_scalar_mul(
                    ctx_row[0:1, ts(ec, ECW)], ps_c[:], rc[0:1, 0:1]
                )

            def finish_ctx(pend):
                bb, wc, rc, ctx_row, done = pend
                for ec in range(done, n_ec):
                    emit_ctx_chunk(bb, wc, rc, ctx_row, ec)
                nc.sync.dma_start(ctx_d.ap()[bb:bb + 1, :], ctx_row[:])
                for tb in range(n_tb):
                    del encnat_all[(bb, tb)]

            for i, (b, tcn) in enumerate(pairs):
                # issue loads two pairs ahead
                nxt2 = i + 2
                if nxt2 < len(pairs):
                    b2, tcn2 = pairs[nxt2]
                    for tbl in range(tb_per_tc):
                        tb2 = tcn2 * tb_per_tc + tbl
                        if (b2, tb2) not in encnat_all:
                            load_enc(b2, tb2)
                if i == 0:
                    encT_cur = [emit_transposes(b, tcn, eb) for eb in range(n_eb)]
                if tcn == 0:
                    scores_rows[b] = soft_p.tile(
                        [1, S_], F32, tag="scores", name=f"scores{b}"
                    )
                scores_row = scores_rows[b]

                ps_s = psS.tile([1, TCW], F32, tag="s", name=f"pss{b}_{tcn}")
                encT_next = []
                prev_th = None

                def emit_score(ab_prev, th_prev, last):
                    nc.tensor.matmul(
                        ps_s[:],
                        vcol[:, ab_prev:ab_prev + 1],
                        th_prev[:],
                        start=(ab_prev == 0),
                        stop=last,
                        skip_group_check=True,
                    )

                for ab in range(n_ab):
                    if i == 0:
                        ensure_w(min(ab // AB_GRP + 1, n_abg - 1))
                        ensure_w(ab // AB_GRP)
                    ps_e = psE.tile([P, TCW], F32, tag="e", name=f"pse{b}_{tcn}_{ab}")
                    for eb in range(n_eb):
                        nc.tensor.matmul(
                            ps_e[:],
                            wenc_t[(eb, ab // AB_GRP)][:, ts(ab % AB_GRP, P)],
                            encT_cur[eb][:],
                            start=(eb == 0),
                            stop=(eb == n_eb - 1),
                        )
                    th = tanh_p.tile([P, TCW], F32R, tag="tanh", name=f"th{b}_{tcn}_{ab}")
                    nc.scalar.activation(
                        th[:], ps_e[:], AF.Tanh,
                        bias=hb_all[:, ab * B_loc + b: ab * B_loc + b + 1],
                    )
                    # scores matmul for the PREVIOUS a-block (one block of
                    # energy work between tanh and its consumer)
                    if prev_th is not None:
                        emit_score(ab - 1, prev_th, False)
                    prev_th = th
                    # interleave next pair's transposes
                    if ab < n_eb and i + 1 < len(pairs):
                        bn, tcnn = pairs[i + 1]
                        encT_next.append(emit_transposes(bn, tcnn, ab))
                    # interleave previous batch's context matmuls
                    if pending_ctx is not None and ab >= 2 and pending_ctx[4] < n_ec:
                        bb, wc, rc, ctx_row, done = pending_ctx
                        emit_ctx_chunk(bb, wc, rc, ctx_row, done)
                        pending_ctx = (bb, wc, rc, ctx_row, done + 1)
                emit_score(n_ab - 1, prev_th, True)
                nc.vector.tensor_copy(scores_row[0:1, ts(tcn, TCW)], ps_s[:])
                encT_cur = encT_next

                if pending_ctx is not None and pending_ctx[4] >= n_ec:
                    bb, wc, rc, ctx_row, done = pending_ctx
                    nc.sync.dma_start(ctx_d.ap()[bb:bb + 1, :], ctx_row[:])
                    for tb in range(n_tb):
                        del encnat_all[(bb, tb)]
                    pending_ctx = None

                if tcn == n_tc - 1:
                    # ---- per-batch softmax; ctx matmuls deferred ----
                    row = scores_row[0:1, :]
                    nm = soft_p.tile([1, 1], F32, tag="nm", name=f"nm{b}")
                    nc.vector.reduce_max(nm[:], row, axis=AX.X, negate=True)
                    ew = soft_p.tile([1, S_], F32, tag="ew", name=f"ew{b}")
                    nc.scalar.activation(ew[:], row, AF.Exp, bias=nm[0:1, 0:1])
                    sm = soft_p.tile([1, 1], F32, tag="sm", name=f"sm{b}")
                    nc.vector.reduce_sum(sm[:], ew[:], axis=AX.X)
                    rc = soft_p.tile([1, 1], F32, tag="rc", name=f"rc{b}")
                    nc.vector.reciprocal(rc[:], sm[:])
                    # re-lay ew into PE columns via rank-1 matmuls (no DMA)
                    pswc = psC.tile([P, n_tb], F32, tag="c", name=f"pswc{b}")
                    for tb in range(n_tb):
                        nc.tensor.matmul(
                            pswc[:, tb:tb + 1],
                            ew[0:1, ts(tb, P)],
                            ones1[:],
                            start=True,
                            stop=True,
                            skip_group_check=True,
                        )
                    wc = soft_p.tile([P, n_tb], F32R, tag="wc", name=f"wc{b}")
                    nc.vector.tensor_copy(wc[:], pswc[:])
                    ctx_row = soft_p.tile([1, E], F32, tag="ctxrow", name=f"ctxrow{b}")
                    if pending_ctx is not None:
                        finish_ctx(pending_ctx)
                    pending_ctx = (b, wc, rc, ctx_row, 0)

            if pending_ctx is not None:
                finish_ctx(pending_ctx)

    nc.compile()
    return nc


def _get_nc():
    key = (B_LOC, S, ENC, ATTN, DEC)
    if key not in _CACHE:
        _CACHE[key] = build_nc(*key)
    return _CACHE[key]


def kernel(hidden, encoder_outputs, W_attn, b_attn, v):
    from concourse.bass_utils import run_bass_kernel_spmd

    hidden = np.ascontiguousarray(np.asarray(hidden, dtype=np.float32))
    enc = np.ascontiguousarray(np.asarray(encoder_outputs, dtype=np.float32))
    W = np.ascontiguousarray(np.asarray(W_attn, dtype=np.float32))
    b = np.ascontiguousarray(np.asarray(b_attn, dtype=np.float32))
    vv = np.ascontiguousarray(np.asarray(v, dtype=np.float32))

    nc = _get_nc()
    in_maps = [
        {
            "hidden": hidden[c * B_LOC:(c + 1) * B_LOC],
            "enc": enc[c * B_LOC:(c + 1) * B_LOC],
            "W": W,
            "b_attn": b,
            "v": vv,
        }
        for c in range(N_CORES)
    ]
    res = run_bass_kernel_spmd(nc, in_maps, core_ids=list(range(N_CORES)))
    out = np.concatenate([res.results[c]["ctx"] for c in range(N_CORES)], axis=0)
    return out.reshape(B, 1, ENC).astype(np.float32)



# revision 18
# speedup vs baseline: 1.1035x; 1.1035x over previous
"""Bass/Trainium2 kernel for the additive-attention nn.Module.

Computation (per batch b):
    energy[s, a] = tanh( enc[b,s,:] @ W_enc[a,:] + hidden[b,:] @ W_dec[a,:] + b_attn[a] )
    scores[s]    = energy[s, :] @ v
    w            = softmax(scores)
    ctx[b, :]    = w @ enc[b]

Sharding: data-parallel over batch across 8 NeuronCores (8 batches/core);
weights replicated.

v2 layout strategy: the host pre-transposes enc ([B, E, S]) and W ([F, A]),
so the device performs ZERO layout transposes for the big GEMM:
  - energy^T runs as psum[a-block, s-chunk] += W_encT[e, a-block].T @
    encT[e, s-chunk]; both operands are direct DMA slices of the host
    layouts (fp32r, 1 PE cycle/row).
  - hb[a, b] = hidden @ W_dec + b_attn is a tiny [8, A] PE GEMM, transposed
    on-chip (8 small PE transposes) into per-partition bias columns.
  - ScalarEngine computes tanh(energy + hb[:, b]) straight from PSUM in one
    pass (per-partition bias), PE contracts with v into scores rows.
  - softmax on the [1, S] scores row (ACT exp with accum_out sum).
  - ctx = Σ_s w_s enc[s, e] is a free-axis reduce over the encT tiles:
    Pool multiplies by the broadcast exp-weight row, DVE reduce_sum gives
    ctxT columns; scaled by 1/sum at the tail, PE-transposed to [b, e] rows.
PE work is ~89% the irreducible energy GEMM.
"""

import sys

if "/opt/trn_rl_repo" not in sys.path:
    sys.path.insert(0, "/opt/trn_rl_repo")

import numpy as np

B, S, DEC, ENC, ATTN = 64, 1024, 1024, 1024, 1024
N_CORES = 8
B_LOC = B // N_CORES

_CACHE = {}


def build_nc(B_loc=B_LOC, S_=S, E=ENC, A=ATTN, D=DEC, loop_n=None):
    from contextlib import ExitStack

    import concourse.bacc as bacc
    import concourse.tile as tile
    from concourse import mybir
    from concourse.bass import ts
    from concourse.masks import make_identity

    P = 128
    F32 = mybir.dt.float32
    F32R = mybir.dt.float32r
    AF = mybir.ActivationFunctionType
    AX = mybir.AxisListType
    ALU = mybir.AluOpType

    n_ab = A // P             # a-blocks (energy psum partition dim)
    n_eb = E // P             # e-blocks (contraction)
    n_db = D // P
    SCW = 512                 # s-chunk width (fp32 moving max)
    n_sc = S_ // SCW

    nc = bacc.Bacc("TRN2", target_bir_lowering=False, debug=False)
    encT_d = nc.dram_tensor("encT", [B_loc, E, S_], F32, kind="ExternalInput")
    we_d = nc.dram_tensor("wenc", [E, A], F32, kind="ExternalInput")
    wd_d = nc.dram_tensor("wdec", [D, A], F32, kind="ExternalInput")
    hidT_d = nc.dram_tensor("hidT", [D, B_loc], F32, kind="ExternalInput")
    b_d = nc.dram_tensor("b_attn", [A], F32, kind="ExternalInput")
    v_d = nc.dram_tensor("v", [A], F32, kind="ExternalInput")
    ctx_d = nc.dram_tensor("ctx", [B_loc, E], F32, kind="ExternalOutput")

    with tile.TileContext(nc) as tc:
        with ExitStack() as ctx:
            const = ctx.enter_context(tc.tile_pool(name="const", bufs=1))
            wenc_p = ctx.enter_context(tc.tile_pool(name="wenc", bufs=n_eb))
            wdec_p = ctx.enter_context(tc.tile_pool(name="wdec", bufs=3))
            encT_p = ctx.enter_context(tc.tile_pool(name="encT", bufs=2 * n_eb))
            th_p = ctx.enter_context(tc.tile_pool(name="th", bufs=4))
            cx_p = ctx.enter_context(tc.tile_pool(name="cx", bufs=2))
            soft_p = ctx.enter_context(tc.tile_pool(name="soft", bufs=2))
            ewbc_p = ctx.enter_context(tc.tile_pool(name="ewbc", bufs=2))
            psE = ctx.enter_context(tc.tile_pool(name="psE", bufs=4, space="PSUM"))
            psS = ctx.enter_context(tc.tile_pool(name="psS", bufs=2, space="PSUM"))
            psH = ctx.enter_context(tc.tile_pool(name="psH", bufs=1, space="PSUM"))

            if loop_n is not None:
                ctx.enter_context(tc.For_i(0, loop_n, 1))

            # ---- small DMAs + constants ----
            hidT = const.tile([P, n_db, B_loc], F32R, name="hidT")
            nc.sync.dma_start(
                hidT[:],
                hidT_d.ap().rearrange("(db p) b -> p db b", p=P).bitcast(F32R),
            )
            b_row = const.tile([1, A], F32, name="b_row")
            nc.sync.dma_start(b_row[:], b_d.ap().rearrange("(o a) -> o a", o=1))
            vcol = const.tile([P, n_ab], F32R, name="vcol")
            nc.sync.dma_start(
                vcol[:], v_d.ap().rearrange("(j p) -> p j", p=P).bitcast(F32R)
            )
            ones8 = const.tile([1, B_loc], F32, name="ones8")
            nc.gpsimd.memset(ones8[:], 1.0)
            ident0 = const.tile([P, P], F32, name="ident0")
            make_identity(nc, ident0[:])
            ident = const.tile([P, P], F32R, name="ident")
            nc.vector.tensor_copy(ident[:], ident0[:])

            # ---- hb = hidden @ W_dec + b_attn -> [B_loc, A] psum, then
            # transpose into per-partition bias columns hb_all[:, ab, b].
            # Emitted before the bulk W_enc/encT DMAs so W_dec lands first
            # and the PE-stream-head hb matmuls don't delay the energy GEMM.
            ph = psH.tile([B_loc, A], F32, tag="ph", name="ph")
            for db in range(n_db):
                wd = wdec_p.tile([P, A], F32R, tag="wd", name=f"wd{db}")
                nc.sync.dma_start(wd[:], wd_d.ap()[ts(db, P), :].bitcast(F32R))
                for ac in range(A // SCW):
                    nc.tensor.matmul(
                        ph[:, ts(ac, SCW)],
                        hidT[:, db],
                        wd[:, ts(ac, SCW)],
                        start=(db == 0),
                        stop=False,
                        skip_group_check=True,
                    )
            for ac in range(A // SCW):
                nc.tensor.matmul(
                    ph[:, ts(ac, SCW)],
                    ones8[:],
                    b_row[0:1, ts(ac, SCW)],
                    start=False,
                    stop=True,
                    skip_group_check=True,
                )
            hb8 = const.tile([B_loc, A], F32R, name="hb8")
            nc.vector.tensor_copy(hb8[:], ph[:])
            hb_all = const.tile([P, n_ab, B_loc], F32, name="hb_all")
            for ab in range(n_ab):
                pt = psS.tile([P, B_loc], F32R, tag="s", name=f"phb{ab}")
                nc.tensor.transpose(
                    pt[:], hb8[:, ts(ab, P)], ident[0:B_loc, 0:B_loc]
                )
                nc.vector.tensor_copy(hb_all[:, ab], pt[:].bitcast(F32))

            # ---- bulk DMAs: W_enc tiles + first batch's encT tiles ----
            we_t = []
            enc_tiles = {}

            def load_enc(b):
                for eb in range(n_eb):
                    t = encT_p.tile([P, S_], F32R, tag="encT", name=f"enc_{b}_{eb}")
                    nc.sync.dma_start(
                        t[:], encT_d.ap()[b, ts(eb, P), :].bitcast(F32R)
                    )
                    enc_tiles[(b, eb)] = t

            for eb in range(n_eb):
                t = wenc_p.tile([P, A], F32R, tag="we", name=f"we{eb}")
                nc.sync.dma_start(t[:], we_d.ap()[ts(eb, P), :].bitcast(F32R))
                we_t.append(t)
            load_enc(0)

            # persistent ctxT columns [e-block, b] + per-batch 1/sum row
            ctxT = [
                const.tile([P, B_loc], F32, name=f"ctxT{eb}") for eb in range(n_eb)
            ]
            rc_row = const.tile([1, B_loc], F32, name="rc_row")

            # ---- per-batch softmax + context (emitted one batch behind) ----
            scores_rows = {}

            def emit_softmax_ctx(b):
                row = scores_rows[b][0:1, :]
                nm = soft_p.tile([1, 1], F32, tag="nm", name=f"nm{b}")
                nc.vector.reduce_max(nm[:], row, axis=AX.X, negate=True)
                ew_row = soft_p.tile([1, S_], F32, tag="ew", name=f"ew{b}")
                esum = soft_p.tile([1, 1], F32, tag="esum", name=f"esum{b}")
                nc.scalar.activation(
                    ew_row[:], row, AF.Exp, bias=nm[0:1, 0:1], accum_out=esum[:],
                )
                nc.vector.reciprocal(rc_row[0:1, b:b + 1], esum[:])
                ew_bc = ewbc_p.tile([P, S_], F32, tag="ewbc", name=f"ewbc{b}")
                nc.gpsimd.partition_broadcast(ew_bc[:], ew_row[:])
                # ctx: mult (split Pool/DVE) + DVE reduce per e-block
                for eb in range(n_eb):
                    cx = cx_p.tile([P, S_], F32, tag="cx", name=f"cx{b}_{eb}")
                    eng = nc.gpsimd if eb % 2 == 0 else nc.vector
                    eng.tensor_tensor(
                        cx[:], enc_tiles[(b, eb)][:].bitcast(F32), ew_bc[:],
                        op=ALU.mult,
                    )
                    nc.vector.reduce_sum(ctxT[eb][:, b:b + 1], cx[:], axis=AX.X)
                for eb in range(n_eb):
                    del enc_tiles[(b, eb)]

            # ---- main loop: energy GEMM + tanh + scores, pipelined ----
            pending_soft = None
            for b in range(B_loc):
                scores_row = soft_p.tile([1, S_], F32, tag="scores", name=f"scores{b}")
                scores_rows[b] = scores_row
                for sc in range(n_sc):
                    ps_s = psS.tile([1, SCW], F32, tag="s", name=f"pss{b}_{sc}")
                    prev = None  # (ab, th) whose score matmul is pending

                    def emit_score(ab, th, last):
                        nc.tensor.matmul(
                            ps_s[:],
                            vcol[:, ab:ab + 1],
                            th[:],
                            start=(ab == 0),
                            stop=last,
                            skip_group_check=True,
                        )

                    for ab in range(n_ab):
                        ps = psE.tile([P, SCW], F32, tag="e", name=f"pse{b}_{sc}_{ab}")
                        for eb in range(n_eb):
                            nc.tensor.matmul(
                                ps[:],
                                we_t[eb][:, ts(ab, P)],
                                enc_tiles[(b, eb)][:, ts(sc, SCW)],
                                start=(eb == 0),
                                stop=(eb == n_eb - 1),
                            )
                        th = th_p.tile([P, SCW], F32R, tag="th", name=f"th{b}_{sc}_{ab}")
                        nc.scalar.activation(
                            th[:], ps[:], AF.Tanh,
                            bias=hb_all[:, ab, b:b + 1],
                        )
                        if prev is not None:
                            emit_score(prev[0], prev[1], False)
                        prev = (ab, th)
                        # previous batch's softmax+ctx + next batch prefetch,
                        # one energy group into this batch
                        if sc == 0 and ab == 1:
                            if pending_soft is not None:
                                emit_softmax_ctx(pending_soft)
                                pending_soft = None
                            if b + 1 < B_loc:
                                load_enc(b + 1)
                    emit_score(prev[0], prev[1], True)
                    nc.vector.tensor_copy(scores_row[0:1, ts(sc, SCW)], ps_s[:])
                pending_soft = b
            if pending_soft is not None:
                emit_softmax_ctx(pending_soft)

            # ---- tail: scale ctxT by 1/sum, transpose to [b, e] rows, DMA ----
            rc_bc = const.tile([P, B_loc], F32, name="rc_bc")
            nc.gpsimd.partition_broadcast(rc_bc[:], rc_row[:])
            crows = const.tile([B_loc, E], F32, name="crows")
            for eb in range(n_eb):
                cts = soft_p.tile([P, B_loc], F32R, tag="cts", name=f"cts{eb}")
                nc.vector.tensor_tensor(cts[:], ctxT[eb][:], rc_bc[:], op=ALU.mult)
                ctr = psS.tile([B_loc, P], F32R, tag="s", name=f"ctr{eb}")
                nc.tensor.transpose(ctr[:], cts[:], ident[:])
                nc.vector.tensor_copy(crows[:, ts(eb, P)], ctr[:].bitcast(F32))
            nc.sync.dma_start(ctx_d.ap(), crows[:])

    nc.compile()
    return nc


def _get_nc():
    key = (B_LOC, S, ENC, ATTN, DEC)
    if key not in _CACHE:
        _CACHE[key] = build_nc(*key)
    return _CACHE[key]


def _prep(hidden, encoder_outputs, W_attn, b_attn, v):
    hidden = np.asarray(hidden, dtype=np.float32)
    enc = np.asarray(encoder_outputs, dtype=np.float32)
    W = np.asarray(W_attn, dtype=np.float32)
    b = np.ascontiguousarray(np.asarray(b_attn, dtype=np.float32))
    vv = np.ascontiguousarray(np.asarray(v, dtype=np.float32))

    encT = np.ascontiguousarray(enc.transpose(0, 2, 1))          # [B, E, S]
    WT = np.ascontiguousarray(W.T)                               # [F, A]
    wdec = np.ascontiguousarray(WT[:DEC])                        # [D, A]
    wenc = np.ascontiguousarray(WT[DEC:])                        # [E, A]
    hidT = np.ascontiguousarray(hidden.T)                        # [D, B]
    return encT, wenc, wdec, hidT, b, vv


def kernel(hidden, encoder_outputs, W_attn, b_attn, v):
    from concourse.bass_utils import run_bass_kernel_spmd

    encT, wenc, wdec, hidT, b, vv = _prep(
        hidden, encoder_outputs, W_attn, b_attn, v
    )

    nc = _get_nc()
    in_maps = [
        {
            "encT": encT[c * B_LOC:(c + 1) * B_LOC],
            "wenc": wenc,
            "wdec": wdec,
            "hidT": np.ascontiguousarray(hidT[:, c * B_LOC:(c + 1) * B_LOC]),
            "b_attn": b,
            "v": vv,
        }
        for c in range(N_CORES)
    ]
    res = run_bass_kernel_spmd(nc, in_maps, core_ids=list(range(N_CORES)))
    out = np.concatenate([res.results[c]["ctx"] for c in range(N_CORES)], axis=0)
    return out.reshape(B, 1, ENC).astype(np.float32)


# revision 22
# speedup vs baseline: 1.2561x; 1.1383x over previous
"""Bass/Trainium2 kernel for the additive-attention nn.Module.

Computation (per batch b):
    energy[s, a] = tanh( enc[b,s,:] @ W_enc[a,:] + hidden[b,:] @ W_dec[a,:] + b_attn[a] )
    scores[s]    = energy[s, :] @ v
    w            = softmax(scores)
    ctx[b, :]    = w @ enc[b]

Sharding: data-parallel over batch across 8 NeuronCores (8 batches/core);
weights replicated.

v2 layout strategy: the host pre-transposes enc ([B, E, S]) and W ([F, A]),
so the device performs ZERO layout transposes for the big GEMM:
  - energy^T runs as psum[a-block, s-chunk] += W_encT[e, a-block].T @
    encT[e, s-chunk]; both operands are direct DMA slices of the host
    layouts (fp32r, 1 PE cycle/row).
  - hb[a, b] = hidden @ W_dec + b_attn is a tiny [8, A] PE GEMM, transposed
    on-chip (8 small PE transposes) into per-partition bias columns.
  - ScalarEngine computes tanh(energy + hb[:, b]) straight from PSUM in one
    pass (per-partition bias), PE contracts with v into scores rows.
  - softmax on the [1, S] scores row (ACT exp with accum_out sum).
  - ctx = Σ_s w_s enc[s, e] is a free-axis reduce over the encT tiles:
    Pool multiplies by the broadcast exp-weight row, DVE reduce_sum gives
    ctxT columns; scaled by 1/sum at the tail, PE-transposed to [b, e] rows.
PE work is ~89% the irreducible energy GEMM.
"""

import sys

if "/opt/trn_rl_repo" not in sys.path:
    sys.path.insert(0, "/opt/trn_rl_repo")

import numpy as np

B, S, DEC, ENC, ATTN = 64, 1024, 1024, 1024, 1024
N_CORES = 8
B_LOC = B // N_CORES

_CACHE = {}


def build_nc(B_loc=B_LOC, S_=S, E=ENC, A=ATTN, D=DEC, loop_n=None):
    from contextlib import ExitStack

    import concourse.bacc as bacc
    import concourse.tile as tile
    from concourse import mybir
    from concourse.bass import ts
    from concourse.masks import make_identity

    P = 128
    F32 = mybir.dt.float32
    F32R = mybir.dt.float32r
    AF = mybir.ActivationFunctionType
    AX = mybir.AxisListType
    ALU = mybir.AluOpType

    n_ab = A // P             # a-blocks (energy psum partition dim)
    n_eb = E // P             # e-blocks (contraction)
    n_db = D // P
    SCW = 512                 # s-chunk width (fp32 moving max)
    n_sc = S_ // SCW

    nc = bacc.Bacc("TRN2", target_bir_lowering=False, debug=False)
    encT_d = nc.dram_tensor("encT", [B_loc, E, S_], F32, kind="ExternalInput")
    we_d = nc.dram_tensor("wenc", [E, A], F32, kind="ExternalInput")
    wd_d = nc.dram_tensor("wdec", [D, A], F32, kind="ExternalInput")
    hidT_d = nc.dram_tensor("hidT", [D, B_loc], F32, kind="ExternalInput")
    b_d = nc.dram_tensor("b_attn", [A], F32, kind="ExternalInput")
    v_d = nc.dram_tensor("v", [A], F32, kind="ExternalInput")
    ctx_d = nc.dram_tensor("ctx", [B_loc, E], F32, kind="ExternalOutput")

    with tile.TileContext(nc) as tc:
        with ExitStack() as ctx:
            const = ctx.enter_context(tc.tile_pool(name="const", bufs=1))
            wenc_p = ctx.enter_context(tc.tile_pool(name="wenc", bufs=n_eb))
            wdec_p = ctx.enter_context(tc.tile_pool(name="wdec", bufs=n_db))
            encT_p = ctx.enter_context(tc.tile_pool(name="encT", bufs=2 * n_eb))
            th_p = ctx.enter_context(tc.tile_pool(name="th", bufs=4))
            cx_p = ctx.enter_context(tc.tile_pool(name="cx", bufs=2))
            soft_p = ctx.enter_context(tc.tile_pool(name="soft", bufs=2))
            ewbc_p = ctx.enter_context(tc.tile_pool(name="ewbc", bufs=2))
            psE = ctx.enter_context(tc.tile_pool(name="psE", bufs=5, space="PSUM"))
            psS = ctx.enter_context(tc.tile_pool(name="psS", bufs=2, space="PSUM"))

            if loop_n is not None:
                ctx.enter_context(tc.For_i(0, loop_n, 1))

            # ---- small DMAs + constants ----
            hidT = const.tile([P, n_db, B_loc], F32R, name="hidT")
            nc.sync.dma_start(
                hidT[:],
                hidT_d.ap().rearrange("(db p) b -> p db b", p=P).bitcast(F32R),
            )
            b_row = const.tile([1, A], F32, name="b_row")
            nc.sync.dma_start(b_row[:], b_d.ap().rearrange("(o a) -> o a", o=1))
            vcol = const.tile([P, n_ab], F32R, name="vcol")
            nc.sync.dma_start(
                vcol[:], v_d.ap().rearrange("(j p) -> p j", p=P).bitcast(F32R)
            )
            ones8 = const.tile([1, B_loc], F32, name="ones8")
            nc.gpsimd.memset(ones8[:], 1.0)
            ident0 = const.tile([P, P], F32, name="ident0")
            make_identity(nc, ident0[:])
            ident = const.tile([P, P], F32R, name="ident")
            nc.vector.tensor_copy(ident[:], ident0[:])

            # ---- hb = hidden @ W_dec + b_attn -> [B_loc, A] psum, then
            # transpose into per-partition bias columns hb_all[:, ab, b].
            # Emitted before the bulk W_enc/encT DMAs so W_dec lands first
            # and the PE-stream-head hb matmuls don't delay the energy GEMM.
            hb8 = const.tile([B_loc, A], F32R, name="hb8")
            wd_tiles = {}
            for db in range(n_db):
                wd = wdec_p.tile([P, A], F32R, tag="wd", name=f"wd{db}")
                nc.sync.dma_start(wd[:], wd_d.ap()[ts(db, P), :].bitcast(F32R))
                wd_tiles[db] = wd
            for ac in range(A // SCW):
                ph = psS.tile([B_loc, SCW], F32, tag="s", name=f"ph{ac}")
                for db in range(n_db):
                    nc.tensor.matmul(
                        ph[:],
                        hidT[:, db],
                        wd_tiles[db][:, ts(ac, SCW)],
                        start=(db == 0),
                        stop=False,
                        skip_group_check=True,
                    )
                nc.tensor.matmul(
                    ph[:],
                    ones8[:],
                    b_row[0:1, ts(ac, SCW)],
                    start=False,
                    stop=True,
                    skip_group_check=True,
                )
                nc.vector.tensor_copy(hb8[:, ts(ac, SCW)], ph[:])
            hb_all = const.tile([P, n_ab, B_loc], F32, name="hb_all")
            for ab in range(n_ab):
                pt = psS.tile([P, B_loc], F32R, tag="s", name=f"phb{ab}")
                nc.tensor.transpose(
                    pt[:], hb8[:, ts(ab, P)], ident[0:B_loc, 0:B_loc]
                )
                nc.vector.tensor_copy(hb_all[:, ab], pt[:].bitcast(F32))

            # ---- bulk DMAs: W_enc tiles + first batch's encT tiles ----
            we_t = []
            enc_tiles = {}

            def load_enc(b):
                for eb in range(n_eb):
                    t = encT_p.tile([P, S_], F32R, tag="encT", name=f"enc_{b}_{eb}")
                    nc.sync.dma_start(
                        t[:], encT_d.ap()[b, ts(eb, P), :].bitcast(F32R)
                    )
                    enc_tiles[(b, eb)] = t

            for eb in range(n_eb):
                t = wenc_p.tile([P, A], F32R, tag="we", name=f"we{eb}")
                nc.sync.dma_start(t[:], we_d.ap()[ts(eb, P), :].bitcast(F32R))
                we_t.append(t)
            load_enc(0)

            # persistent ctxT columns [e-block, b] + per-batch 1/sum row
            ctxT = [
                const.tile([P, B_loc], F32, name=f"ctxT{eb}") for eb in range(n_eb)
            ]
            rc_row = const.tile([1, B_loc], F32, name="rc_row")

            # ---- per-batch softmax + context (emitted one batch behind) ----
            scores_rows = {}

            def emit_softmax_ctx(b):
                row = scores_rows[b][0:1, :]
                nm = soft_p.tile([1, 1], F32, tag="nm", name=f"nm{b}")
                nc.vector.reduce_max(nm[:], row, axis=AX.X, negate=True)
                ew_row = soft_p.tile([1, S_], F32, tag="ew", name=f"ew{b}")
                esum = soft_p.tile([1, 1], F32, tag="esum", name=f"esum{b}")
                nc.scalar.activation(
                    ew_row[:], row, AF.Exp, bias=nm[0:1, 0:1], accum_out=esum[:],
                )
                nc.vector.reciprocal(rc_row[0:1, b:b + 1], esum[:])
                ew_bc = ewbc_p.tile([P, S_], F32, tag="ewbc", name=f"ewbc{b}")
                nc.gpsimd.partition_broadcast(ew_bc[:], ew_row[:])
                # ctx: mult (split Pool/DVE) + DVE reduce per e-block
                for eb in range(n_eb):
                    cx = cx_p.tile([P, S_], F32, tag="cx", name=f"cx{b}_{eb}")
                    eng = nc.gpsimd if eb % 2 == 0 else nc.vector
                    eng.tensor_tensor(
                        cx[:], enc_tiles[(b, eb)][:].bitcast(F32), ew_bc[:],
                        op=ALU.mult,
                    )
                    nc.vector.reduce_sum(ctxT[eb][:, b:b + 1], cx[:], axis=AX.X)
                for eb in range(n_eb):
                    del enc_tiles[(b, eb)]

            # ---- main loop: energy GEMM + tanh + scores, pipelined ----
            pending_soft = None
            for b in range(B_loc):
                scores_row = soft_p.tile([1, S_], F32, tag="scores", name=f"scores{b}")
                scores_rows[b] = scores_row
                for sc in range(n_sc):
                    ps_s = psS.tile([1, SCW], F32, tag="s", name=f"pss{b}_{sc}")
                    prev = None  # (ab, th) whose score matmul is pending

                    def emit_score(ab, th, last):
                        nc.tensor.matmul(
                            ps_s[:],
                            vcol[:, ab:ab + 1],
                            th[:],
                            start=(ab == 0),
                            stop=last,
                            skip_group_check=True,
                        )

                    for ab in range(n_ab):
                        ps = psE.tile([P, SCW], F32, tag="e", name=f"pse{b}_{sc}_{ab}")
                        for eb in range(n_eb):
                            nc.tensor.matmul(
                                ps[:],
                                we_t[eb][:, ts(ab, P)],
                                enc_tiles[(b, eb)][:, ts(sc, SCW)],
                                start=(eb == 0),
                                stop=(eb == n_eb - 1),
                            )
                        th = th_p.tile([P, SCW], F32R, tag="th", name=f"th{b}_{sc}_{ab}")
                        nc.scalar.activation(
                            th[:], ps[:], AF.Tanh,
                            bias=hb_all[:, ab, b:b + 1],
                        )
                        if prev is not None:
                            emit_score(prev[0], prev[1], False)
                        prev = (ab, th)
                        # previous batch's softmax+ctx + next batch prefetch,
                        # one energy group into this batch
                        if sc == 0 and ab == 1:
                            if pending_soft is not None:
                                emit_softmax_ctx(pending_soft)
                                pending_soft = None
                            if b + 1 < B_loc:
                                load_enc(b + 1)
                    emit_score(prev[0], prev[1], True)
                    nc.vector.tensor_copy(scores_row[0:1, ts(sc, SCW)], ps_s[:])
                pending_soft = b
            if pending_soft is not None:
                emit_softmax_ctx(pending_soft)

            # ---- tail: scale ctxT by 1/sum, transpose to [b, e] rows, DMA ----
            rc_bc = const.tile([P, B_loc], F32, name="rc_bc")
            nc.gpsimd.partition_broadcast(rc_bc[:], rc_row[:])
            crows = const.tile([B_loc, E], F32, name="crows")
            for eb in range(n_eb):
                cts = soft_p.tile([P, B_loc], F32R, tag="cts", name=f"cts{eb}")
                nc.vector.tensor_tensor(cts[:], ctxT[eb][:], rc_bc[:], op=ALU.mult)
                ctr = psS.tile([B_loc, P], F32R, tag="s", name=f"ctr{eb}")
                nc.tensor.transpose(ctr[:], cts[:], ident[:])
                nc.vector.tensor_copy(crows[:, ts(eb, P)], ctr[:].bitcast(F32))
            nc.sync.dma_start(ctx_d.ap(), crows[:])

    nc.compile()
    return nc


def _get_nc():
    key = (B_LOC, S, ENC, ATTN, DEC)
    if key not in _CACHE:
        _CACHE[key] = build_nc(*key)
    return _CACHE[key]


def _prep(hidden, encoder_outputs, W_attn, b_attn, v):
    hidden = np.asarray(hidden, dtype=np.float32)
    enc = np.asarray(encoder_outputs, dtype=np.float32)
    W = np.asarray(W_attn, dtype=np.float32)
    b = np.ascontiguousarray(np.asarray(b_attn, dtype=np.float32))
    vv = np.ascontiguousarray(np.asarray(v, dtype=np.float32))

    encT = np.ascontiguousarray(enc.transpose(0, 2, 1))          # [B, E, S]
    WT = np.ascontiguousarray(W.T)                               # [F, A]
    wdec = np.ascontiguousarray(WT[:DEC])                        # [D, A]
    wenc = np.ascontiguousarray(WT[DEC:])                        # [E, A]
    hidT = np.ascontiguousarray(hidden.T)                        # [D, B]
    return encT, wenc, wdec, hidT, b, vv


def kernel(hidden, encoder_outputs, W_attn, b_attn, v):
    from concourse.bass_utils import run_bass_kernel_spmd

    encT, wenc, wdec, hidT, b, vv = _prep(
        hidden, encoder_outputs, W_attn, b_attn, v
    )

    nc = _get_nc()
    in_maps = [
        {
            "encT": encT[c * B_LOC:(c + 1) * B_LOC],
            "wenc": wenc,
            "wdec": wdec,
            "hidT": np.ascontiguousarray(hidT[:, c * B_LOC:(c + 1) * B_LOC]),
            "b_attn": b,
            "v": vv,
        }
        for c in range(N_CORES)
    ]
    res = run_bass_kernel_spmd(nc, in_maps, core_ids=list(range(N_CORES)))
    out = np.concatenate([res.results[c]["ctx"] for c in range(N_CORES)], axis=0)
    return out.reshape(B, 1, ENC).astype(np.float32)
